# revision 1
# baseline (speedup 1.0000x reference)
"""BertSelfAttention with gated prompt-prefix branch on 8 Trainium2 cores.

Sharding: data-parallel over batch (B=8 -> 1 batch element per core), no
collectives. Per core, the full attention pipeline runs in a transposed
[feature, seq] layout so that softmax statistics ride through the matmuls:

  qT/kT = W @ hsT          [768, 1024]  (bf16, PE)
  v_aug = hs @ WvT_aug     [1024, 780]  natural layout, 65-col stride per
                           head, col 65h+64 = ones (denominator column)
  scoresT_h = kh @ qh.T    [t, s] via K=64 row-tiled matmuls, 2 heads
                           concurrently on the 128x128 PE array
  expT = exp(SCALE*scoresT + mask[t])   one fused ACT op per tile
  ctxT_aug_h = v_aug_h.T @ expT_h       rows 0..63 ctx, row 64 = sum_t exp
  prefix branch identical with prompt-derived k/v; tanh(gate) folded into
  the prefix v weights on-device
  out_h = ctxT/denom + pctxT/pdenom     (DVE, reciprocal + partition bcast)

Output is produced as outT [768, 1024] fp32 per core; the host transposes
and stacks to [8, 1024, 768].
"""

import numpy as np
import ml_dtypes

import concourse.bass as bass
import concourse.mybir as mybir
import concourse.tile as tile
from concourse.bass_utils import run_bass_kernel_spmd
from concourse.vector_clock import ScopedClock


class SplitDrainTileContext(tile.TileContext):
    """This walrus build rejects >2 sync waits on the kernel-tail Drain
    ("Too many sync wait commands"); split them across SP nops instead."""

    def _drain_and_barrier(self, tick_clock, wait_clock):
        probe = self.nc.sync.nop(nofuse=True, hint="drain_wait_split")
        wait_clock.add_sem_waits(
            probe.ins, ScopedClock({None: tick_clock.global_clock})
        )
        waits = list(probe.ins.sync_info.on_wait or [])
        if len(waits) > 1:
            probe.ins.sync_info.on_wait = waits[:1]
            for i in range(1, len(waits)):
                extra = self.nc.sync.nop(nofuse=True, hint="drain_wait_split")
                extra.ins.sync_info = mybir.SyncInfo(
                    on_wait=waits[i : i + 1], on_update=[]
                )
        drain_inst = self.nc.sync.drain()
        if drain_inst.ins.sync_info is not None:
            drain_inst.ins.sync_info.on_wait = []
        self.nc.all_engine_barrier()
        assert self.sems is not None
        popped = self.nc._tile_sem_poison_stack.pop()
        assert popped is self._sem_poison
        self.nc.clear_and_free_semaphores(list(self.sems.allocated().values()))
        self.nc.all_engine_barrier()

F32 = mybir.dt.float32
BF16 = mybir.dt.bfloat16
AF = mybir.ActivationFunctionType

H, DH, D = 12, 64, 768
S, AT, B = 1024, 64, 8
SCALE = 1.0 / np.sqrt(DH)
NC_D = D // 128  # 6 chunks over feature dim
NC_S = S // 128  # 8 chunks over sequence dim
PAIRS = H // 2  # 6 head pairs
VW = H * (DH + 1)  # 780: v with per-head ones column

_CACHE = {}
LAST_RESULTS = None


def _split_sync_waits(nc, cap=1):
    """Walrus on this image allows very few sync-wait commands per
    instruction (tensor_scalar rejects 2). Hoist excess waits onto
    same-engine nops placed immediately before the instruction."""
    for bb in nc.main_func.blocks:
        cur = list(bb.instructions)
        out = []
        for inst in cur:
            si = inst.sync_info
            waits = list(si.on_wait) if si and si.on_wait else []
            if len(waits) > cap:
                for i in range(0, len(waits) - cap):
                    bi = nc.engines[inst.engine].nop(
                        nofuse=True, hint="wait_split")
                    popped = nc.cur_bb.bb.instructions.pop()
                    assert popped is bi.ins
                    bi.ins.sync_info = mybir.SyncInfo(
                        on_wait=waits[i : i + 1], on_update=[])
                    out.append(bi.ins)
                si.on_wait = waits[len(waits) - cap:]
            out.append(inst)
        bb.instructions[:] = out


def _build_nc():
    nc = bass.Bass()
    hsT = nc.dram_tensor("hsT", [D, S], BF16, kind="ExternalInput")
    wqT = nc.dram_tensor("wqT", [D, D], BF16, kind="ExternalInput")
    wkT = nc.dram_tensor("wkT", [D, D], BF16, kind="ExternalInput")
    wvT = nc.dram_tensor("wvT", [D, VW], BF16, kind="ExternalInput")
    bq = nc.dram_tensor("bq", [D, 1], F32, kind="ExternalInput")
    bk = nc.dram_tensor("bk", [D, 1], F32, kind="ExternalInput")
    bvaug = nc.dram_tensor("bvaug", [128, VW], F32, kind="ExternalInput")
    promptT = nc.dram_tensor("promptT", [D, AT], BF16, kind="ExternalInput")
    mask = nc.dram_tensor("mask", [S, 1], F32, kind="ExternalInput")
    gating = nc.dram_tensor("gating", [128, VW], F32, kind="ExternalInput")
    outT = nc.dram_tensor("outT", [D, S], F32, kind="ExternalOutput")

    with SplitDrainTileContext(nc) as tc:
        _emit(nc, tc, hsT, wqT, wkT, wvT, bq, bk, bvaug, promptT, mask,
              gating, outT)
    _split_sync_waits(nc)
    return nc


def _emit(nc, tc, hsT, wqT, wkT, wvT, bq, bk, bvaug, promptT, mask, gating,
          outT):
    from contextlib import ExitStack

    with ExitStack() as ctx:
        pers = ctx.enter_context(tc.tile_pool(name="pers", bufs=1))

        # ---- SBUF arrays that live into the attention phase ----
        mask_sb = pers.tile([128, NC_S], F32, tag="mask")
        emask_sb = pers.tile([128, NC_S], F32, tag="emask")
        qT_sb = pers.tile([128, NC_D * S], BF16, tag="qT")
        kT_sb = pers.tile([128, NC_D * S], BF16, tag="kT")
        v_sb = pers.tile([128, NC_S * VW], BF16, tag="v")
        pkT_sb = pers.tile([128, NC_D * AT], BF16, tag="pkT")
        pv_sb = pers.tile([128, VW], BF16, tag="pv")

        # ---- projection-phase-only arrays (pool closed afterwards so the
        # attention pools can reuse the space) ----
        proj_cm = tc.tile_pool(name="proj", bufs=1, side="right")
        proj = proj_cm.__enter__()
        hsT_sb = proj.tile([128, NC_D * S], BF16, tag="hsT")
        wqT_sb = proj.tile([128, NC_D * D], BF16, tag="wqT")
        wkT_sb = proj.tile([128, NC_D * D], BF16, tag="wkT")
        wvT_sb = proj.tile([128, NC_D * VW], BF16, tag="wvT")
        pT_sb = proj.tile([128, NC_D * AT], BF16, tag="pT")
        bq_sb = proj.tile([128, NC_D], F32, tag="bq")
        bk_sb = proj.tile([128, NC_D], F32, tag="bk")
        bvaug_sb = proj.tile([128, VW], F32, tag="bvaug")
        graw_sb = proj.tile([128, VW], F32, tag="graw")
        gbc_sb = proj.tile([128, VW], F32, tag="gbc")
        pvtmp_sb = proj.tile([64, VW], F32, tag="pvtmp")

        for src, dst, w in ((wqT, wqT_sb, D), (wkT, wkT_sb, D),
                            (hsT, hsT_sb, S), (wvT, wvT_sb, VW),
                            (promptT, pT_sb, AT)):
            nc.sync.dma_start(
                dst[:].rearrange("p (c s) -> p c s", s=w),
                src[:, :].rearrange("(c p) s -> p c s", p=128))
        # biases / mask: [768,1] & [1024,1] -> [128, nchunks]
        nc.sync.dma_start(bq_sb[:], bq.rearrange("(c p) 1 -> p c", p=128))
        nc.sync.dma_start(bk_sb[:], bk.rearrange("(c p) 1 -> p c", p=128))
        nc.sync.dma_start(mask_sb[:], mask.rearrange("(c p) 1 -> p c", p=128))
        nc.sync.dma_start(bvaug_sb[:], bvaug[:])
        # gating arrives host-replicated to [128, 780] (65 copies per head
        # along the row, broadcast down the partitions)
        nc.sync.dma_start(graw_sb[:], gating[:])
        # tanh, then force the ones-column slots back to 1.0
        nc.scalar.activation(gbc_sb[:], graw_sb[:], AF.Tanh)
        ones_slots = gbc_sb[:, :].rearrange(
            "p (h e) -> p h e", h=H)[:, :, DH:DH + 1]
        nc.vector.memset(ones_slots, 1.0)
        # e^mask, folded into the V rows (incl. ones column) instead of an
        # exp bias: exp(S*x + m_t) == e^{m_t} * exp(S*x), and the ones
        # column then accumulates the correctly-masked denominator.
        nc.scalar.activation(emask_sb[:], mask_sb[:], AF.Exp)

        # SBUF pools that outlive the projection phase — opened before the
        # closeable PSUM pools so the per-side pool stack unwinds LIFO
        exp_pool = ctx.enter_context(tc.tile_pool(name="expp", bufs=4))
        pexp_pool = ctx.enter_context(tc.tile_pool(name="pexpp", bufs=3))

        # ---- PSUM pool for the projection phase (closed afterwards) ----
        mm_cm = tc.tile_pool(name="mm", bufs=2, space="PSUM")
        mm_pool = mm_cm.__enter__()

        # ---- Q/K projections (transposed layout) ----
        for c in range(NC_D):
            for w_sb, b_sb, o_sb in ((wqT_sb, bq_sb, qT_sb),
                                     (wkT_sb, bk_sb, kT_sb)):
                ps = mm_pool.tile([128, S], F32, tag="mm")
                for kc in range(NC_D):
                    lhsT = w_sb[:, kc * D + c * 128: kc * D + (c + 1) * 128]
                    for sb in range(2):
                        nc.tensor.matmul(
                            ps[:, sb * 512:(sb + 1) * 512], lhsT,
                            hsT_sb[:, kc * S + sb * 512: kc * S + (sb + 1) * 512],
                            start=(kc == 0), stop=(kc == NC_D - 1))
                nc.vector.tensor_scalar_add(o_sb[:, c * S:(c + 1) * S],
                                            ps[:], b_sb[:, c:c + 1])

        # PSUM banks 4-7 (on top of mm's 0-3); closed before mm so the
        # pool stack unwinds LIFO, then reopened for pairs 1..5
        sc0_cm = tc.tile_pool(name="scp0", bufs=2, space="PSUM")
        scp = {"p": sc0_cm.__enter__()}

        def prefix_scores(c, pexp):
            sc_pool = scp["p"]
            psp = sc_pool.tile([128, S], F32, tag="sc", name=f"psp_{c}")
            for half in range(2):
                hp = half * 64
                for sb in range(2):
                    nc.tensor.matmul(
                        psp[hp:hp + 64, sb * 512:(sb + 1) * 512],
                        pkT_sb[hp:hp + 64, c * AT:(c + 1) * AT],
                        qT_sb[hp:hp + 64,
                              c * S + sb * 512: c * S + (sb + 1) * 512],
                        tile_position=(hp, hp))
            nc.scalar.activation(pexp[:], psp[:], AF.Exp, scale=SCALE)

        def scores_exp(c, exp_ab, pexp, ctx_mms=None):
            sc_pool = scp["p"]
            """Scores + exp for pair c ([t,s] layout, 2 heads row-tiled);
            optionally interleaves ctx matmuls for chunk tci-1 to keep the
            PE dense."""
            for tci in range(NC_S):
                for half in range(2):
                    hp = half * 64
                    st = sc_pool.tile([128, S], F32, tag="sc",
                                      name=f"st_{c}_{tci}_{half}")
                    lhsT = kT_sb[hp:hp + 64,
                                 c * S + tci * 128: c * S + (tci + 1) * 128]
                    for sb in range(2):
                        nc.tensor.matmul(
                            st[:, sb * 512:(sb + 1) * 512], lhsT,
                            qT_sb[hp:hp + 64,
                                  c * S + sb * 512: c * S + (sb + 1) * 512],
                            tile_position=(hp, 0))
                    nc.scalar.activation(
                        exp_ab[half][:, tci * S:(tci + 1) * S],
                        st[:], AF.Exp, scale=SCALE)
                if tci == 0:
                    if pexp is not None:
                        # prefix scores ride in the first bubble
                        prefix_scores(c, pexp)
                elif ctx_mms is not None:
                    for half in range(2):
                        ctx_mms(half, tci - 1)

        # pair-0 scores start as soon as qT/kT chunk 0 exists, overlapping
        # the V/prompt projections below (ACT would otherwise sit idle).
        # The prefix part waits until pkT exists, in finish_pair(0).
        exp0 = [exp_pool.tile([128, NC_S * S], BF16, tag="exp",
                              name=f"exp_0_{i}") for i in range(2)]
        pexp0 = pexp_pool.tile([128, S], BF16, tag="pexp", name="pexp0")
        scores_exp(0, exp0, None)

        # ---- V projection (natural layout, augmented ones column) ----
        for sc in range(NC_S):
            ps = mm_pool.tile([128, S], F32, tag="mm")
            for kc in range(NC_D):
                lhsT = hsT_sb[:, kc * S + sc * 128: kc * S + (sc + 1) * 128]
                nc.tensor.matmul(ps[:, 0:512], lhsT,
                                 wvT_sb[:, kc * VW: kc * VW + 512],
                                 start=(kc == 0), stop=(kc == NC_D - 1))
                nc.tensor.matmul(ps[:, 512:VW], lhsT,
                                 wvT_sb[:, kc * VW + 512: (kc + 1) * VW],
                                 start=(kc == 0), stop=(kc == NC_D - 1))
            vt = proj.tile([128, VW], F32, tag="vtmp", name=f"vt{sc}",
                           bufs=2)
            nc.vector.tensor_add(vt[:], ps[:, 0:VW], bvaug_sb[:])
            nc.vector.tensor_scalar_mul(v_sb[:, sc * VW:(sc + 1) * VW],
                                        vt[:], emask_sb[:, sc:sc + 1])

        # ---- prompt K projection (transposed) ----
        for c in range(NC_D):
            ps = mm_pool.tile([128, S], F32, tag="mm")
            for kc in range(NC_D):
                nc.tensor.matmul(
                    ps[:, 0:AT],
                    wkT_sb[:, kc * D + c * 128: kc * D + (c + 1) * 128],
                    pT_sb[:, kc * AT:(kc + 1) * AT],
                    start=(kc == 0), stop=(kc == NC_D - 1))
            nc.vector.tensor_scalar_add(pkT_sb[:, c * AT:(c + 1) * AT],
                                        ps[:, 0:AT], bk_sb[:, c:c + 1])

        # ---- prompt V projection (natural, gate-scaled, duplicated) ----
        ps = mm_pool.tile([128, S], F32, tag="mm")
        for kc in range(NC_D):
            lhsT = pT_sb[:, kc * AT:(kc + 1) * AT]
            nc.tensor.matmul(ps[0:AT, 0:512], lhsT,
                             wvT_sb[:, kc * VW: kc * VW + 512],
                             start=(kc == 0), stop=(kc == NC_D - 1))
            nc.tensor.matmul(ps[0:AT, 512:VW], lhsT,
                             wvT_sb[:, kc * VW + 512: (kc + 1) * VW],
                             start=(kc == 0), stop=(kc == NC_D - 1))
        nc.vector.tensor_add(pvtmp_sb[:], ps[0:AT, 0:VW], bvaug_sb[0:AT, :])
        nc.vector.tensor_mul(pv_sb[0:AT, :], pvtmp_sb[:], gbc_sb[0:AT, :])
        nc.sync.dma_start(pv_sb[AT:128, :], pv_sb[0:AT, :])

        sc0_cm.__exit__(None, None, None)
        proj_cm.__exit__(None, None, None)
        mm_cm.__exit__(None, None, None)

        # ---- remaining attention pools (reuse the projection PSUM) ----
        scp["p"] = ctx.enter_context(
            tc.tile_pool(name="scp", bufs=2, space="PSUM"))
        ctx_pool = ctx.enter_context(
            tc.tile_pool(name="ctxp", bufs=2, space="PSUM"))
        norm_pool = ctx.enter_context(tc.tile_pool(name="normp", bufs=2))
        out_pool = ctx.enter_context(tc.tile_pool(name="outp", bufs=2))
        dscr_pool = ctx.enter_context(
            tc.tile_pool(name="dscr", bufs=2, space="DRAM"))

        def make_ctx_mms(c, cps_ab, exp_ab):
            def ctx_mms(half, tci):
                h = 2 * c + half
                lhsT = v_sb[:, tci * VW + h * 65: tci * VW + h * 65 + 65]
                for sb in range(2):
                    nc.tensor.matmul(
                        cps_ab[half][:, sb * 512:(sb + 1) * 512], lhsT,
                        exp_ab[half][:, tci * S + sb * 512:
                                     tci * S + (sb + 1) * 512],
                        start=(tci == 0), stop=(tci == NC_S - 1))
            return ctx_mms

        def finish_stage1(c, exp_ab, pexp, cps_ab):
            """Prefix ctx matmuls, psum evacuation (frees banks fast), and
            the denominator reciprocal/broadcast chain. The slow combine
            (waits on the broadcast DMA) is deferred to finish_stage2 so it
            never blocks the DVE FIFO ahead of psum-freeing copies."""
            state = []
            for half in range(2):
                h = 2 * c + half
                hp = half * 64
                cps = cps_ab[half]
                pps = scp["p"].tile([128, S], F32, tag="sc",
                                    name=f"pps_{c}_{half}")
                for sb in range(2):
                    nc.tensor.matmul(
                        pps[0:65, sb * 512:(sb + 1) * 512],
                        pv_sb[hp:hp + 64, h * 65: h * 65 + 65],
                        pexp[hp:hp + 64, sb * 512:(sb + 1) * 512],
                        tile_position=(hp, 0))

                ce = norm_pool.tile([65, S], F32, tag="ce", bufs=4,
                                    name=f"ce_{c}_{half}")
                pe_ev = norm_pool.tile([65, S], F32, tag="pe_ev", bufs=4,
                                       name=f"pe_{c}_{half}")
                nc.vector.tensor_copy(ce[:], cps[:])
                nc.vector.tensor_copy(pe_ev[:], pps[0:65, :])
                # denominator rows -> DMA-reshape across partitions ->
                # cheap wide reciprocal -> DRAM -> broadcast
                dresh = norm_pool.tile([128, 16], F32, tag="dresh", bufs=4,
                                       name=f"dr_{c}_{half}")
                nc.sync.dma_start(dresh[:, 0:8], ce[64:65, :])
                nc.sync.dma_start(dresh[:, 8:16], pe_ev[64:65, :])
                rrec = norm_pool.tile([128, 16], F32, tag="rrec", bufs=4,
                                      name=f"rr_{c}_{half}")
                nc.vector.reciprocal(rrec[:], dresh[:])
                r_d = dscr_pool.tile([1, 2 * S], F32, tag="rd", bufs=4,
                                     name=f"rd_{c}_{half}")
                nc.sync.dma_start(r_d[0:1, 0:S], rrec[:, 0:8])
                nc.sync.dma_start(r_d[0:1, S:2 * S], rrec[:, 8:16])
                r_bc = norm_pool.tile([64, 2 * S], F32, tag="rbc", bufs=4,
                                      name=f"rbc_{c}_{half}")
                r_src = bass.AP(r_d[:].tensor, r_d[:].offset,
                                [[0, 64], [1, 2 * S]])
                nc.sync.dma_start(r_bc[:], r_src)
                state.append((h, ce, pe_ev, r_bc))
            return state

        def finish_stage2(c, state):
            for h, ce, pe_ev, r_bc in state:
                # normalize in place, combine on GpSimd
                nc.vector.tensor_mul(ce[0:64, :], ce[0:64, :], r_bc[:, 0:S])
                nc.vector.tensor_mul(pe_ev[0:64, :], pe_ev[0:64, :],
                                     r_bc[:, S:2 * S])
                ot = out_pool.tile([64, S], F32, tag="ot",
                                   name=f"ot_{c}_{h}")
                nc.gpsimd.tensor_add(ot[:], ce[0:64, :], pe_ev[0:64, :])
                nc.sync.dma_start(outT[h * 64:(h + 1) * 64, :], ot[:])

        # pair 0: ctx for the pre-computed exps, then the remaining pairs
        # with ctx interleaved behind their own score/exp stream
        cps0 = [ctx_pool.tile([65, S], F32, tag="ctx", name=f"cps_0_{i}")
                for i in range(2)]
        ctx0 = make_ctx_mms(0, cps0, exp0)
        prefix_scores(0, pexp0)
        for tci in range(NC_S):
            for half in range(2):
                ctx0(half, tci)
        pending = (0, finish_stage1(0, exp0, pexp0, cps0))

        for c in range(1, PAIRS):
            exp_ab = [exp_pool.tile([128, NC_S * S], BF16, tag="exp",
                                    name=f"exp_{c}_{i}")
                      for i in range(2)]
            pexp = pexp_pool.tile([128, S], BF16, tag="pexp",
                                  name=f"pexp_{c}")
            cps_ab = [ctx_pool.tile([65, S], F32, tag="ctx",
                                    name=f"cps_{c}_{i}")
                      for i in range(2)]
            cmm = make_ctx_mms(c, cps_ab, exp_ab)
            scores_exp(c, exp_ab, pexp, ctx_mms=cmm)
            finish_stage2(*pending)
            for half in range(2):
                cmm(half, NC_S - 1)
            pending = (c, finish_stage1(c, exp_ab, pexp, cps_ab))
        finish_stage2(*pending)


def _prep_inputs(hidden_states, prompt_tokens, gating_factor, attention_mask,
                 Wq, bq, Wk, bk, Wv, bv):
    bf = ml_dtypes.bfloat16
    hs = np.asarray(hidden_states, np.float32)
    mask = np.asarray(attention_mask, np.float32).reshape(B, S)
    wqT = np.ascontiguousarray(np.asarray(Wq, np.float32).T).astype(bf)
    wkT = np.ascontiguousarray(np.asarray(Wk, np.float32).T).astype(bf)
    # augmented WvT: [din, 780], col 65h+j = Wv.T[:, 64h+j], col 65h+64 = 0
    wvT_f = np.asarray(Wv, np.float32).T  # [din, dout]
    wvT_aug = np.zeros((D, VW), np.float32)
    idx = np.arange(D)
    aug_cols = (idx // DH) * (DH + 1) + (idx % DH)
    wvT_aug[:, aug_cols] = wvT_f
    wvT_aug = wvT_aug.astype(bf)
    bq_c = np.asarray(bq, np.float32).reshape(D, 1)
    bk_c = np.asarray(bk, np.float32).reshape(D, 1)
    bv_aug = np.zeros(VW, np.float32)
    bv_aug[aug_cols] = np.asarray(bv, np.float32)
    bv_aug[DH::DH + 1] = 1.0
    bvaug_bc = np.ascontiguousarray(
        np.broadcast_to(bv_aug, (128, VW)), np.float32)
    pT = np.ascontiguousarray(
        np.asarray(prompt_tokens, np.float32)[0].T).astype(bf)
    gat_row = np.repeat(
        np.asarray(gating_factor, np.float32).reshape(H), DH + 1)
    gat = np.ascontiguousarray(
        np.broadcast_to(gat_row, (128, VW)), np.float32)

    shared = dict(wqT=wqT, wkT=wkT, wvT=wvT_aug, bq=bq_c, bk=bk_c,
                  bvaug=bvaug_bc, promptT=pT, gating=gat)
    in_maps = []
    for b in range(B):
        m = dict(shared)
        m["hsT"] = np.ascontiguousarray(hs[b].T).astype(bf)
        m["mask"] = np.ascontiguousarray(mask[b].reshape(S, 1))
        in_maps.append(m)
    return in_maps


def kernel(**inputs):
    global LAST_RESULTS
    if "nc" not in _CACHE:
        _CACHE["nc"] = _build_nc()
    nc = _CACHE["nc"]
    in_maps = _prep_inputs(**inputs)
    res = None
    for attempt in range(3):
        try:
            res = run_bass_kernel_spmd(nc, in_maps, list(range(B)))
            break
        except ModuleNotFoundError:
            # BASS_TRACE set but this image lacks antenv.axon_hooks
            import os

            os.environ["BASS_NEVER_TRACE"] = "1"
            if attempt == 2:
                raise
        except Exception:
            # transient NRT_EXEC_UNIT_UNRECOVERABLE on a cold device has
            # been observed; a retry on the same session recovers
            if attempt == 2:
                raise
    LAST_RESULTS = res
    out = np.empty((B, S, D), np.float32)
    for b in range(B):
        out[b] = res.results[b]["outT"].T
    return out



# revision 5
# speedup vs baseline: 1.1343x; 1.1343x over previous
"""BertSelfAttention with gated prompt-prefix branch on 8 Trainium2 cores.

Sharding: data-parallel over batch (B=8 -> 1 batch element per core), no
collectives. Head-granular pipeline per core:

  Q/K proj (bf16, K=128)  -> fp8 q8/k8 in DoubleRow slab layout
  scores_h = k8.T @ q8    fp8 DoubleRow (K=32x2 slots), [t,s], 2x col rate
  exp = exp(SCALE*score)  ACT, bf16 out [t, s]
  ctx accumulates NATURAL [s, d]: stationary = exp[t-block, s-block],
  rhs = v_aug[t-block, 65] (col 64 = ones*e^mask -> denominator lands in
  psum col 64, PER-PARTITION in s)
  prefix branch identical from prompt-derived pk8/pv (tanh(gate) folded
  into pv on-device)
  finish: DVE reciprocal + broadcast-mul + scalar_tensor_tensor, all
  per-partition; output written natural [1024, 768] f32 (no transpose)
"""

import numpy as np
import ml_dtypes

import concourse.bass as bass
import concourse.mybir as mybir
import concourse.tile as tile
from concourse.bass_utils import run_bass_kernel_spmd
from concourse.vector_clock import ScopedClock


class SplitDrainTileContext(tile.TileContext):
    """This walrus build rejects >2 sync waits on the kernel-tail Drain
    ("Too many sync wait commands"); split them across SP nops instead."""

    def _drain_and_barrier(self, tick_clock, wait_clock):
        probe = self.nc.sync.nop(nofuse=True, hint="drain_wait_split")
        wait_clock.add_sem_waits(
            probe.ins, ScopedClock({None: tick_clock.global_clock})
        )
        waits = list(probe.ins.sync_info.on_wait or [])
        if len(waits) > 1:
            probe.ins.sync_info.on_wait = waits[:1]
            for i in range(1, len(waits)):
                extra = self.nc.sync.nop(nofuse=True, hint="drain_wait_split")
                extra.ins.sync_info = mybir.SyncInfo(
                    on_wait=waits[i : i + 1], on_update=[]
                )
        drain_inst = self.nc.sync.drain()
        if drain_inst.ins.sync_info is not None:
            drain_inst.ins.sync_info.on_wait = []
        self.nc.all_engine_barrier()
        assert self.sems is not None
        popped = self.nc._tile_sem_poison_stack.pop()
        assert popped is self._sem_poison
        self.nc.clear_and_free_semaphores(list(self.sems.allocated().values()))
        self.nc.all_engine_barrier()

F32 = mybir.dt.float32
BF16 = mybir.dt.bfloat16
FP8 = mybir.dt.float8e4
AF = mybir.ActivationFunctionType
ALU = mybir.AluOpType
DR = mybir.MatmulPerfMode.DoubleRow

H, DH, D = 12, 64, 768
S, AT, B = 1024, 64, 8
SCALE = 1.0 / np.sqrt(DH)
NC_D = D // 128  # 6 contraction chunks
NC_S = S // 128  # 8 sequence chunks
VW = H * (DH + 1)  # 780: v with per-head ones column

_CACHE = {}
LAST_RESULTS = None


def _split_sync_waits(nc, cap=1):
    """Walrus on this image allows very few sync-wait commands per
    instruction (tensor_scalar rejects 2). Hoist excess waits onto
    same-engine nops placed immediately before the instruction."""
    for bb in nc.main_func.blocks:
        cur = list(bb.instructions)
        out = []
        for inst in cur:
            si = inst.sync_info
            waits = list(si.on_wait) if si and si.on_wait else []
            if len(waits) > cap:
                for i in range(0, len(waits) - cap):
                    bi = nc.engines[inst.engine].nop(
                        nofuse=True, hint="wait_split")
                    popped = nc.cur_bb.bb.instructions.pop()
                    assert popped is bi.ins
                    bi.ins.sync_info = mybir.SyncInfo(
                        on_wait=waits[i : i + 1], on_update=[])
                    out.append(bi.ins)
                si.on_wait = waits[len(waits) - cap:]
            out.append(inst)
        bb.instructions[:] = out


def _build_nc():
    nc = bass.Bass()
    hsT = nc.dram_tensor("hsT", [D, S], BF16, kind="ExternalInput")
    wqT = nc.dram_tensor("wqT", [D, D], BF16, kind="ExternalInput")
    wkT = nc.dram_tensor("wkT", [D, D], BF16, kind="ExternalInput")
    wvT = nc.dram_tensor("wvT", [D, VW], BF16, kind="ExternalInput")
    bq = nc.dram_tensor("bq", [D, 1], F32, kind="ExternalInput")
    bk = nc.dram_tensor("bk", [D, 1], F32, kind="ExternalInput")
    bvaug = nc.dram_tensor("bvaug", [128, VW], F32, kind="ExternalInput")
    promptT = nc.dram_tensor("promptT", [D, AT], BF16, kind="ExternalInput")
    mask = nc.dram_tensor("mask", [S, 1], F32, kind="ExternalInput")
    gating = nc.dram_tensor("gating", [128, VW], F32, kind="ExternalInput")
    out_nat = nc.dram_tensor("out_nat", [S, D], F32, kind="ExternalOutput")

    with SplitDrainTileContext(nc) as tc:
        _emit(nc, tc, hsT, wqT, wkT, wvT, bq, bk, bvaug, promptT, mask,
              gating, out_nat)
    _split_sync_waits(nc)
    return nc


def _emit(nc, tc, hsT, wqT, wkT, wvT, bq, bk, bvaug, promptT, mask, gating,
          out_nat):
    from contextlib import ExitStack

    with ExitStack() as ctx:
        pers = ctx.enter_context(tc.tile_pool(name="pers", bufs=1))

        # ---- persistent SBUF ----
        hs_k = [pers.tile([128, S], BF16, tag=f"hs{k}", name=f"hs{k}")
                for k in range(NC_D)]
        wq_c = [pers.tile([128, D], BF16, tag=f"wq{c}", name=f"wq{c}")
                for c in range(NC_D)]
        wk_c = [pers.tile([128, D], BF16, tag=f"wk{c}", name=f"wk{c}")
                for c in range(NC_D)]
        wv_k = [pers.tile([128, VW], BF16, tag=f"wv{k}", name=f"wv{k}")
                for k in range(NC_D)]
        pT_sb = pers.tile([128, NC_D * AT], BF16, tag="pT")
        bq_sb = pers.tile([128, NC_D], F32, tag="bq")
        bk_sb = pers.tile([128, NC_D], F32, tag="bk")
        bvaug_sb = pers.tile([128, VW], F32, tag="bvaug")
        graw_sb = pers.tile([128, VW], F32, tag="graw")
        gbc_sb = pers.tile([128, VW], F32, tag="gbc")
        mask_sb = pers.tile([128, NC_S], F32, tag="mask")
        emask_sb = pers.tile([128, NC_S], F32, tag="emask")
        # fp8 slab layout: head h at partitions [32*(h%4), +32),
        # free offset 2048*(h//4) + slot*1024 + s   (d_local = 32*slot + r)
        q8_sb = pers.tile([128, 3 * 2 * S], FP8, tag="q8")
        k8_sb = pers.tile([128, 3 * 2 * S], FP8, tag="k8")
        # prefix operands, natural d-on-partition layout (DoubleRow cannot
        # target dst col 64, so prefix scores run as plain fp8 matmuls):
        # head h at partitions [64*(h%2), +64), free AT*(h//2) / S*(h//2)
        pk8f_sb = pers.tile([128, NC_D * AT], FP8, tag="pk8f")
        q8f_sb = pers.tile([128, NC_D * S], FP8, tag="q8f")
        v_sb = pers.tile([128, NC_S * VW], BF16, tag="v")
        pv_sb = pers.tile([128, VW], BF16, tag="pv")

        # ---- rotating SBUF pools ----
        exp_pool = ctx.enter_context(tc.tile_pool(name="expp", bufs=3))
        pexp_pool = ctx.enter_context(tc.tile_pool(name="pexpp", bufs=2))
        stage_pool = ctx.enter_context(tc.tile_pool(name="stgp", bufs=3))
        vt_pool = ctx.enter_context(tc.tile_pool(name="vtp", bufs=2))
        out_pool = ctx.enter_context(tc.tile_pool(name="outp", bufs=2))
        r_pool = ctx.enter_context(tc.tile_pool(name="rp", bufs=2))

        # ---- PSUM: pool4 = 2 tiles x [128,1024] (4 banks);
        #      ctx_pool = 4 tiles x [128,512] (4 banks) ----
        pool4 = ctx.enter_context(
            tc.tile_pool(name="p4", bufs=2, space="PSUM"))
        ctx_pool = ctx.enter_context(
            tc.tile_pool(name="ctxp", bufs=4, space="PSUM"))

        # ---- input DMAs, priority order ----
        for k in range(NC_D):
            nc.sync.dma_start(
                hs_k[k][:], hsT[k * 128:(k + 1) * 128, :])
        for c in (0,):
            nc.sync.dma_start(
                wq_c[c][:].rearrange("p (k n) -> p k n", n=128),
                wqT[:, c * 128:(c + 1) * 128].rearrange(
                    "(k p) n -> p k n", p=128))
            nc.sync.dma_start(
                wk_c[c][:].rearrange("p (k n) -> p k n", n=128),
                wkT[:, c * 128:(c + 1) * 128].rearrange(
                    "(k p) n -> p k n", p=128))
        nc.sync.dma_start(bq_sb[:], bq.rearrange("(c p) 1 -> p c", p=128))
        nc.sync.dma_start(bk_sb[:], bk.rearrange("(c p) 1 -> p c", p=128))
        nc.sync.dma_start(mask_sb[:], mask.rearrange("(c p) 1 -> p c", p=128))
        nc.sync.dma_start(bvaug_sb[:], bvaug[:])
        nc.sync.dma_start(graw_sb[:], gating[:])
        nc.sync.dma_start(
            pT_sb[:].rearrange("p (k n) -> p k n", n=AT),
            promptT[:, :].rearrange("(k p) n -> p k n", p=128))
        for k in range(NC_D):
            nc.sync.dma_start(
                wv_k[k][:], wvT[k * 128:(k + 1) * 128, :])
        for c in range(1, NC_D):
            nc.sync.dma_start(
                wq_c[c][:].rearrange("p (k n) -> p k n", n=128),
                wqT[:, c * 128:(c + 1) * 128].rearrange(
                    "(k p) n -> p k n", p=128))
            nc.sync.dma_start(
                wk_c[c][:].rearrange("p (k n) -> p k n", n=128),
                wkT[:, c * 128:(c + 1) * 128].rearrange(
                    "(k p) n -> p k n", p=128))

        # ---- small precompute ----
        nc.scalar.activation(gbc_sb[:], graw_sb[:], AF.Tanh)
        ones_slots = gbc_sb[:, :].rearrange(
            "p (h e) -> p h e", h=H)[:, :, DH:DH + 1]
        nc.vector.memset(ones_slots, 1.0)
        nc.scalar.activation(emask_sb[:], mask_sb[:], AF.Exp)

        # ---- emission helpers ----
        def slab(h):
            # odd heads at PE rows 0/64: walrus rejects 32-row stationaries
            # at (row 32|96, dst col 64) for the odd-half prefix scores
            return 32 * ((h % 4) ^ 1)

        def proj_qk(c):
            for w_c, b_sb, dst8 in ((wq_c, bq_sb, q8_sb),
                                    (wk_c, bk_sb, k8_sb)):
                ps = pool4.tile([128, S], F32, tag="p4",
                                name=f"pqk_{c}_{w_c is wk_c}")
                for kc in range(NC_D):
                    lhsT = w_c[c][:, kc * 128:(kc + 1) * 128]
                    for sb2 in range(2):
                        nc.tensor.matmul(
                            ps[:, sb2 * 512:(sb2 + 1) * 512], lhsT,
                            hs_k[kc][:, sb2 * 512:(sb2 + 1) * 512],
                            start=(kc == 0), stop=(kc == NC_D - 1))
                st8 = stage_pool.tile([128, S], FP8, tag="stg",
                                      name=f"st8_{c}_{w_c is wk_c}")
                nc.vector.tensor_scalar_add(st8[:], ps[:], b_sb[:, c:c + 1])
                for g in range(2):
                    h = 2 * c + g
                    base = slab(h)
                    fo = 2048 * (h // 4)
                    for i in range(2):
                        nc.sync.dma_start(
                            dst8[base:base + 32,
                                 fo + i * 1024:fo + (i + 1) * 1024],
                            st8[64 * g + 32 * i:64 * g + 32 * i + 32, :])
                if dst8 is q8_sb:
                    for g in range(2):
                        nc.sync.dma_start(
                            q8f_sb[64 * g:64 * g + 64, c * S:(c + 1) * S],
                            st8[64 * g:64 * g + 64, :])

        def proj_pk(c):
            ps = pool4.tile([128, S], F32, tag="p4", name=f"ppk_{c}")
            for kc in range(NC_D):
                nc.tensor.matmul(
                    ps[:, 0:AT], wk_c[c][:, kc * 128:(kc + 1) * 128],
                    pT_sb[:, kc * AT:(kc + 1) * AT],
                    start=(kc == 0), stop=(kc == NC_D - 1))
            st8 = stage_pool.tile([128, AT], FP8, tag="pkstg",
                                  name=f"pk8s_{c}")
            nc.vector.tensor_scalar_add(st8[:], ps[:, 0:AT], bk_sb[:, c:c + 1])
            for g in range(2):
                nc.sync.dma_start(
                    pk8f_sb[64 * g:64 * g + 64, c * AT:(c + 1) * AT],
                    st8[64 * g:64 * g + 64, :])

        def proj_v(sc):
            ps = pool4.tile([128, S], F32, tag="p4", name=f"pv_{sc}")
            for kc in range(NC_D):
                lhsT = hs_k[kc][:, sc * 128:(sc + 1) * 128]
                nc.tensor.matmul(ps[:, 0:512], lhsT, wv_k[kc][:, 0:512],
                                 start=(kc == 0), stop=(kc == NC_D - 1))
                nc.tensor.matmul(ps[:, 512:VW], lhsT, wv_k[kc][:, 512:VW],
                                 start=(kc == 0), stop=(kc == NC_D - 1))
            vt = vt_pool.tile([128, VW], F32, tag="vt", name=f"vt{sc}")
            nc.vector.tensor_add(vt[:], ps[:, 0:VW], bvaug_sb[:])
            nc.vector.tensor_scalar_mul(v_sb[:, sc * VW:(sc + 1) * VW],
                                        vt[:], emask_sb[:, sc:sc + 1])

        def proj_pv():
            ps = pool4.tile([128, S], F32, tag="p4", name="ppv")
            for kc in range(NC_D):
                lhsT = pT_sb[:, kc * AT:(kc + 1) * AT]
                nc.tensor.matmul(ps[0:AT, 0:512], lhsT, wv_k[kc][:, 0:512],
                                 start=(kc == 0), stop=(kc == NC_D - 1))
                nc.tensor.matmul(ps[0:AT, 512:VW], lhsT, wv_k[kc][:, 512:VW],
                                 start=(kc == 0), stop=(kc == NC_D - 1))
            pvt = vt_pool.tile([AT, VW], F32, tag="pvt", name="pvt")
            nc.vector.tensor_add(pvt[:], ps[0:AT, 0:VW], bvaug_sb[0:AT, :])
            nc.vector.tensor_mul(pv_sb[0:AT, :], pvt[:], gbc_sb[0:AT, :])
            nc.sync.dma_start(pv_sb[AT:128, :], pv_sb[0:AT, :])

        def q8ap(h, sb2):
            base = slab(h)
            fo = 2048 * (h // 4)
            return q8_sb[base:base + 32, fo:fo + 2048].rearrange(
                "p (i n) -> p i n", i=2)[:, :, sb2 * 512:(sb2 + 1) * 512]

        def scores(h, exp_h):
            base = slab(h)
            fo = 2048 * (h // 4)
            for tci in range(NC_S):
                st = pool4.tile([128, S], F32, tag="p4",
                                name=f"st_{h}_{tci}")
                lhsT = k8_sb[base:base + 32, fo:fo + 2048].rearrange(
                    "p (i n) -> p i n", i=2)[:, :,
                                             tci * 128:(tci + 1) * 128]
                for sb2 in range(2):
                    nc.tensor.matmul(
                        st[:, sb2 * 512:(sb2 + 1) * 512], lhsT, q8ap(h, sb2),
                        start=True, stop=True, perf_mode=DR,
                        tile_position=(base, 0))
                nc.scalar.activation(exp_h[:, tci * S:(tci + 1) * S],
                                     st[:], AF.Exp, scale=SCALE)

        def pfx_scores(c):
            ps = pool4.tile([128, S], F32, tag="p4", name=f"pfs_{c}")
            for g in range(2):
                hp = 64 * g
                lhsT = pk8f_sb[hp:hp + 64, c * AT:(c + 1) * AT]
                for sb2 in range(2):
                    rhs = q8f_sb[hp:hp + 64,
                                 c * S + sb2 * 512:c * S + (sb2 + 1) * 512]
                    nc.tensor.matmul(
                        ps[hp:hp + 64, sb2 * 512:(sb2 + 1) * 512],
                        lhsT, rhs,
                        start=True, stop=True,
                        tile_position=(hp, hp))
            pexp = pexp_pool.tile([128, S], BF16, tag="pexp",
                                  name=f"pexp_{c}")
            nc.scalar.activation(pexp[:], ps[:], AF.Exp, scale=SCALE)
            return pexp

        def ctx_tci(h, tci, exp_h, ctxA, ctxB):
            for sc in range(NC_S):
                lhsT = exp_h[:, tci * S + sc * 128:tci * S + (sc + 1) * 128]
                rhs = v_sb[:, tci * VW + h * 65:tci * VW + h * 65 + 65]
                if sc < 7:
                    out = ctxA[:, sc * 65:(sc + 1) * 65]
                else:
                    out = ctxB[:, 0:65]
                nc.tensor.matmul(
                    out, lhsT, rhs,
                    start=(tci == 0 and sc in (0, 7)),
                    stop=(tci == NC_S - 1 and sc == 6),
                    skip_group_check=True)

        def pfx_ctx(h, pexp, pfxA, ctxB):
            hp = 64 * (h % 2)
            for sc in range(NC_S):
                lhsT = pexp[hp:hp + 64, sc * 128:(sc + 1) * 128]
                rhs = pv_sb[hp:hp + 64, h * 65:h * 65 + 65]
                if sc < 7:
                    out = pfxA[:, sc * 65:(sc + 1) * 65]
                    st_fl, sp_fl = (sc == 0), (sc == 6)
                else:
                    out = ctxB[:, 65:130]
                    st_fl, sp_fl = False, True
                nc.tensor.matmul(out, lhsT, rhs, start=st_fl, stop=sp_fl,
                                 skip_group_check=True,
                                 tile_position=(hp, 0))

        def bcast7(r16, col):
            a = r16[:, col:col + 7]
            return bass.AP(a.tensor, a.offset, [a.ap[0], [1, 7], [0, 64]])

        def strided7(t, off):
            a = t[:]
            return bass.AP(a.tensor, a.offset + off, [a.ap[0], [65, 7]])

        def finish(h, ctxA, ctxB, pfxA):
            r16 = r_pool.tile([128, 16], F32, tag="r16", name=f"r16_{h}")
            cA = ctxA[:, 0:455].rearrange("p (a b) -> p a b", b=65)
            pA = pfxA[:, 0:455].rearrange("p (a b) -> p a b", b=65)
            nc.vector.reciprocal(r16[:, 0:7], strided7(ctxA, 64))
            nc.vector.reciprocal(r16[:, 7:8], ctxB[:, 64:65])
            nc.vector.reciprocal(r16[:, 8:15], strided7(pfxA, 64))
            nc.vector.reciprocal(r16[:, 15:16], ctxB[:, 129:130])
            outb = out_pool.tile([128, 512], F32, tag="ob", name=f"ob_{h}")
            o3 = outb[:].rearrange("p (a b) -> p a b", b=64)
            nc.vector.tensor_mul(o3[:, 0:7, :], cA[:, :, 0:64],
                                 bcast7(r16, 0))
            nc.vector.tensor_scalar_mul(outb[:, 448:512], ctxB[:, 0:64],
                                        r16[:, 7:8])
            tmp = out_pool.tile([128, 448], F32, tag="tmp", name=f"tm_{h}")
            t3 = tmp[:].rearrange("p (a b) -> p a b", b=64)
            nc.vector.tensor_mul(t3[:, :, :], pA[:, :, 0:64], bcast7(r16, 8))
            nc.gpsimd.tensor_add(outb[:, 0:448], outb[:, 0:448], tmp[:])
            nc.vector.scalar_tensor_tensor(
                outb[:, 448:512], ctxB[:, 65:129], r16[:, 15:16],
                outb[:, 448:512], op0=ALU.mult, op1=ALU.add)
            base = out_nat[:, :]
            dst = bass.AP(base.tensor, base.offset + h * 64,
                          [[D, 128], [128 * D, 8], [1, 64]])
            nc.sync.dma_start(dst, o3[:, :, :])

        # ---- master emission sequence ----
        proj_qk(0)
        proj_pk(0)
        proj_v(0)
        proj_v(1)

        pexp_cur = None
        for h in range(H):
            c = h // 2
            exp_h = exp_pool.tile([128, NC_S * S], BF16, tag="exp",
                                  name=f"exp_{h}")
            scores(h, exp_h)
            if h % 2 == 0:
                pexp_cur = pfx_scores(c)
                if c + 1 < NC_D:
                    proj_qk(c + 1)
                    proj_pk(c + 1)
            if h == 0:
                proj_pv()
            ctxA = ctx_pool.tile([128, 512], F32, tag="ctx",
                                 name=f"cA_{h}")
            ctxB = ctx_pool.tile([128, 512], F32, tag="ctx",
                                 name=f"cB_{h}")
            for tci in range(NC_S):
                if h == 0 and tci < 6:
                    proj_v(tci + 2)
                ctx_tci(h, tci, exp_h, ctxA, ctxB)
            pfxA = ctx_pool.tile([128, 512], F32, tag="ctx",
                                 name=f"pA_{h}")
            pfx_ctx(h, pexp_cur, pfxA, ctxB)
            finish(h, ctxA, ctxB, pfxA)


def _prep_inputs(hidden_states, prompt_tokens, gating_factor, attention_mask,
                 Wq, bq, Wk, bk, Wv, bv):
    bf = ml_dtypes.bfloat16
    hs = np.asarray(hidden_states, np.float32)
    mask = np.asarray(attention_mask, np.float32).reshape(B, S)
    wqT = np.ascontiguousarray(np.asarray(Wq, np.float32).T).astype(bf)
    wkT = np.ascontiguousarray(np.asarray(Wk, np.float32).T).astype(bf)
    # augmented WvT: [din, 780], col 65h+j = Wv.T[:, 64h+j], col 65h+64 = 0
    wvT_f = np.asarray(Wv, np.float32).T
    wvT_aug = np.zeros((D, VW), np.float32)
    idx = np.arange(D)
    aug_cols = (idx // DH) * (DH + 1) + (idx % DH)
    wvT_aug[:, aug_cols] = wvT_f
    wvT_aug = wvT_aug.astype(bf)
    bq_c = np.asarray(bq, np.float32).reshape(D, 1)
    bk_c = np.asarray(bk, np.float32).reshape(D, 1)
    bv_aug = np.zeros(VW, np.float32)
    bv_aug[aug_cols] = np.asarray(bv, np.float32)
    bv_aug[DH::DH + 1] = 1.0
    bvaug_bc = np.ascontiguousarray(
        np.broadcast_to(bv_aug, (128, VW)), np.float32)
    pT = np.ascontiguousarray(
        np.asarray(prompt_tokens, np.float32)[0].T).astype(bf)
    gat_row = np.repeat(
        np.asarray(gating_factor, np.float32).reshape(H), DH + 1)
    gat = np.ascontiguousarray(
        np.broadcast_to(gat_row, (128, VW)), np.float32)

    shared = dict(wqT=wqT, wkT=wkT, wvT=wvT_aug, bq=bq_c, bk=bk_c,
                  bvaug=bvaug_bc, promptT=pT, gating=gat)
    in_maps = []
    for b in range(B):
        m = dict(shared)
        m["hsT"] = np.ascontiguousarray(hs[b].T).astype(bf)
        m["mask"] = np.ascontiguousarray(mask[b].reshape(S, 1))
        in_maps.append(m)
    return in_maps


def kernel(**inputs):
    global LAST_RESULTS
    if "nc" not in _CACHE:
        _CACHE["nc"] = _build_nc()
    nc = _CACHE["nc"]
    in_maps = _prep_inputs(**inputs)
    res = None
    for attempt in range(3):
        try:
            res = run_bass_kernel_spmd(nc, in_maps, list(range(B)))
            break
        except ModuleNotFoundError:
            import os

            os.environ["BASS_NEVER_TRACE"] = "1"
            if attempt == 2:
                raise
        except Exception:
            if attempt == 2:
                raise
    LAST_RESULTS = res
    out = np.empty((B, S, D), np.float32)
    for b in range(B):
        out[b] = res.results[b]["out_nat"]
    return out


# revision 15
# speedup vs baseline: 1.4019x; 1.2360x over previous
"""BertSelfAttention with gated prompt-prefix branch on 8 Trainium2 cores.

Sharding: data-parallel over batch (B=8 -> 1 batch element per core), no
collectives. Head-granular pipeline per core:

  Q/K proj (bf16, K=128)  -> fp8 q8/k8 in DoubleRow slab layout
  scores_h = k8.T @ q8    fp8 DoubleRow (K=32x2 slots), [t,s], 2x col rate
  exp = exp(SCALE*score)  ACT, bf16 out [t, s]
  ctx accumulates NATURAL [s, d]: stationary = exp[t-block, s-block],
  rhs = v_aug[t-block, 65] (col 64 = ones*e^mask -> denominator lands in
  psum col 64, PER-PARTITION in s)
  prefix branch identical from prompt-derived pk8/pv (tanh(gate) folded
  into pv on-device)
  finish: DVE reciprocal + broadcast-mul + scalar_tensor_tensor, all
  per-partition; output written natural [1024, 768] f32 (no transpose)
"""

import numpy as np
import ml_dtypes

import concourse.bass as bass
import concourse.mybir as mybir
import concourse.tile as tile
from concourse.bass_utils import run_bass_kernel_spmd
from concourse.vector_clock import ScopedClock


class SplitDrainTileContext(tile.TileContext):
    """This walrus build rejects >2 sync waits on the kernel-tail Drain
    ("Too many sync wait commands"); split them across SP nops instead."""

    def _drain_and_barrier(self, tick_clock, wait_clock):
        probe = self.nc.sync.nop(nofuse=True, hint="drain_wait_split")
        wait_clock.add_sem_waits(
            probe.ins, ScopedClock({None: tick_clock.global_clock})
        )
        waits = list(probe.ins.sync_info.on_wait or [])
        if len(waits) > 1:
            probe.ins.sync_info.on_wait = waits[:1]
            for i in range(1, len(waits)):
                extra = self.nc.sync.nop(nofuse=True, hint="drain_wait_split")
                extra.ins.sync_info = mybir.SyncInfo(
                    on_wait=waits[i : i + 1], on_update=[]
                )
        drain_inst = self.nc.sync.drain()
        if drain_inst.ins.sync_info is not None:
            drain_inst.ins.sync_info.on_wait = []
        self.nc.all_engine_barrier()
        assert self.sems is not None
        popped = self.nc._tile_sem_poison_stack.pop()
        assert popped is self._sem_poison
        self.nc.clear_and_free_semaphores(list(self.sems.allocated().values()))
        self.nc.all_engine_barrier()

F32 = mybir.dt.float32
BF16 = mybir.dt.bfloat16
FP8 = mybir.dt.float8e4
AF = mybir.ActivationFunctionType
ALU = mybir.AluOpType
DR = mybir.MatmulPerfMode.DoubleRow

H, DH, D = 12, 64, 768
S, AT, B = 1024, 64, 8
SCALE = 1.0 / np.sqrt(DH)
NC_D = D // 128  # 6 contraction chunks
NC_S = S // 128  # 8 sequence chunks
VW = H * (DH + 1)  # 780: v with per-head ones column

_CACHE = {}
LAST_RESULTS = None


def _split_sync_waits(nc, cap=1):
    """Walrus on this image allows very few sync-wait commands per
    instruction (tensor_scalar rejects 2). Hoist excess waits onto
    same-engine nops placed immediately before the instruction."""
    for bb in nc.main_func.blocks:
        cur = list(bb.instructions)
        out = []
        for inst in cur:
            si = inst.sync_info
            waits = list(si.on_wait) if si and si.on_wait else []
            if len(waits) > cap:
                for i in range(0, len(waits) - cap):
                    bi = nc.engines[inst.engine].nop(
                        nofuse=True, hint="wait_split")
                    popped = nc.cur_bb.bb.instructions.pop()
                    assert popped is bi.ins
                    bi.ins.sync_info = mybir.SyncInfo(
                        on_wait=waits[i : i + 1], on_update=[])
                    out.append(bi.ins)
                si.on_wait = waits[len(waits) - cap:]
            out.append(inst)
        bb.instructions[:] = out


def _build_nc():
    nc = bass.Bass()
    hsT = nc.dram_tensor("hsT", [D, S], BF16, kind="ExternalInput")
    wqT = nc.dram_tensor("wqT", [D, D], BF16, kind="ExternalInput")
    wkT = nc.dram_tensor("wkT", [D, D], BF16, kind="ExternalInput")
    wvT = nc.dram_tensor("wvT", [D, VW], BF16, kind="ExternalInput")
    bq = nc.dram_tensor("bq", [D, 1], F32, kind="ExternalInput")
    bk = nc.dram_tensor("bk", [D, 1], F32, kind="ExternalInput")
    bvaug = nc.dram_tensor("bvaug", [128, VW], F32, kind="ExternalInput")
    promptT = nc.dram_tensor("promptT", [D, AT], BF16, kind="ExternalInput")
    mask = nc.dram_tensor("mask", [S, 1], F32, kind="ExternalInput")
    gating = nc.dram_tensor("gating", [128, VW], F32, kind="ExternalInput")
    out_nat = nc.dram_tensor("out_nat", [S, D], F32, kind="ExternalOutput")

    with SplitDrainTileContext(nc) as tc:
        _emit(nc, tc, hsT, wqT, wkT, wvT, bq, bk, bvaug, promptT, mask,
              gating, out_nat)
    _split_sync_waits(nc)
    return nc


def _emit(nc, tc, hsT, wqT, wkT, wvT, bq, bk, bvaug, promptT, mask, gating,
          out_nat):
    from contextlib import ExitStack

    with ExitStack() as ctx:
        pers = ctx.enter_context(tc.tile_pool(name="pers", bufs=1))

        # ---- persistent SBUF ----
        hs_k = [pers.tile([128, S], BF16, tag=f"hs{k}", name=f"hs{k}")
                for k in range(NC_D)]
        wq_c = [pers.tile([128, D], BF16, tag=f"wq{c}", name=f"wq{c}")
                for c in range(NC_D)]
        wk_c = [pers.tile([128, D], BF16, tag=f"wk{c}", name=f"wk{c}")
                for c in range(NC_D)]
        wv_k = [pers.tile([128, VW], BF16, tag=f"wv{k}", name=f"wv{k}")
                for k in range(NC_D)]
        pT_sb = pers.tile([128, NC_D * AT], BF16, tag="pT")
        bq_sb = pers.tile([128, NC_D], F32, tag="bq")
        bk_sb = pers.tile([128, NC_D], F32, tag="bk")
        bvaug_sb = pers.tile([128, VW], F32, tag="bvaug")
        graw_sb = pers.tile([128, VW], F32, tag="graw")
        gbc_sb = pers.tile([128, VW], F32, tag="gbc")
        mask_sb = pers.tile([128, NC_S], F32, tag="mask")
        emask_sb = pers.tile([128, NC_S], F32, tag="emask")
        # bf16 q/k in d-major layout straight from the projection psum:
        # chunk c holds heads (2c, 2c+1) on partition halves
        qT_sb = pers.tile([128, NC_D * S], BF16, tag="qT")
        kT_sb = pers.tile([128, NC_D * S], BF16, tag="kT")
        pkT_sb = pers.tile([128, NC_D * AT], BF16, tag="pkT")
        v_sb = pers.tile([128, NC_S * VW], BF16, tag="v")
        pv_sb = pers.tile([128, VW], BF16, tag="pv")

        # ---- rotating SBUF pools ----
        exp_pool = ctx.enter_context(tc.tile_pool(name="expp", bufs=3))
        pexp_pool = ctx.enter_context(tc.tile_pool(name="pexpp", bufs=3))
        vt_pool = ctx.enter_context(tc.tile_pool(name="vtp", bufs=2))
        out_pool = ctx.enter_context(tc.tile_pool(name="outp", bufs=2))
        r_pool = ctx.enter_context(tc.tile_pool(name="rp", bufs=2))

        # ---- PSUM: pool4 = 3 tiles x [128,1024] (6 banks);
        #      ctx_pool = 2 tiles x [128,512] (2 banks) ----
        pool4 = ctx.enter_context(
            tc.tile_pool(name="p4", bufs=3, space="PSUM"))
        ctx_pool = ctx.enter_context(
            tc.tile_pool(name="ctxp", bufs=2, space="PSUM"))

        # ---- input DMAs, priority order ----
        for k in range(NC_D):
            nc.sync.dma_start(
                hs_k[k][:], hsT[k * 128:(k + 1) * 128, :])
        for c in (0,):
            nc.sync.dma_start(
                wq_c[c][:].rearrange("p (k n) -> p k n", n=128),
                wqT[:, c * 128:(c + 1) * 128].rearrange(
                    "(k p) n -> p k n", p=128))
            nc.sync.dma_start(
                wk_c[c][:].rearrange("p (k n) -> p k n", n=128),
                wkT[:, c * 128:(c + 1) * 128].rearrange(
                    "(k p) n -> p k n", p=128))
        nc.sync.dma_start(bq_sb[:], bq.rearrange("(c p) 1 -> p c", p=128))
        nc.sync.dma_start(bk_sb[:], bk.rearrange("(c p) 1 -> p c", p=128))
        nc.sync.dma_start(mask_sb[:], mask.rearrange("(c p) 1 -> p c", p=128))
        nc.sync.dma_start(bvaug_sb[:], bvaug[:])
        nc.sync.dma_start(graw_sb[:], gating[:])
        nc.sync.dma_start(
            pT_sb[:].rearrange("p (k n) -> p k n", n=AT),
            promptT[:, :].rearrange("(k p) n -> p k n", p=128))
        for k in range(NC_D):
            nc.sync.dma_start(
                wv_k[k][:], wvT[k * 128:(k + 1) * 128, :])
        for c in range(1, NC_D):
            nc.sync.dma_start(
                wq_c[c][:].rearrange("p (k n) -> p k n", n=128),
                wqT[:, c * 128:(c + 1) * 128].rearrange(
                    "(k p) n -> p k n", p=128))
            nc.sync.dma_start(
                wk_c[c][:].rearrange("p (k n) -> p k n", n=128),
                wkT[:, c * 128:(c + 1) * 128].rearrange(
                    "(k p) n -> p k n", p=128))

        # ---- small precompute ----
        nc.scalar.activation(gbc_sb[:], graw_sb[:], AF.Tanh)
        ones_slots = gbc_sb[:, :].rearrange(
            "p (h e) -> p h e", h=H)[:, :, DH:DH + 1]
        nc.vector.memset(ones_slots, 1.0)
        nc.scalar.activation(emask_sb[:], mask_sb[:], AF.Exp)

        # ---- emission helpers ----
        def proj_qk(c, which):
            w_c, b_sb, dst = ((wq_c, bq_sb, qT_sb) if which == "q"
                              else (wk_c, bk_sb, kT_sb))
            ps = pool4.tile([128, S], F32, tag="p4", name=f"pqk_{c}_{which}")
            for kc in range(NC_D):
                lhsT = w_c[c][:, kc * 128:(kc + 1) * 128]
                for sb2 in range(2):
                    nc.tensor.matmul(
                        ps[:, sb2 * 512:(sb2 + 1) * 512], lhsT,
                        hs_k[kc][:, sb2 * 512:(sb2 + 1) * 512],
                        start=(kc == 0), stop=(kc == NC_D - 1))
            nc.vector.tensor_scalar_add(dst[:, c * S:(c + 1) * S], ps[:],
                                        b_sb[:, c:c + 1])

        def proj_pk(c):
            ps = pool4.tile([128, S], F32, tag="p4", name=f"ppk_{c}")
            for kc in range(NC_D):
                nc.tensor.matmul(
                    ps[:, 0:AT],
                    wk_c[c][:, kc * 128:(kc + 1) * 128],
                    pT_sb[:, kc * AT:(kc + 1) * AT],
                    start=(kc == 0), stop=(kc == NC_D - 1))
            nc.vector.tensor_scalar_add(pkT_sb[:, c * AT:(c + 1) * AT],
                                        ps[:, 0:AT], bk_sb[:, c:c + 1])

        def proj_v(sc):
            ps = pool4.tile([128, S], F32, tag="p4", name=f"pv_{sc}")
            for kc in range(NC_D):
                lhsT = hs_k[kc][:, sc * 128:(sc + 1) * 128]
                nc.tensor.matmul(ps[:, 0:512], lhsT, wv_k[kc][:, 0:512],
                                 start=(kc == 0), stop=(kc == NC_D - 1))
                nc.tensor.matmul(ps[:, 512:VW], lhsT, wv_k[kc][:, 512:VW],
                                 start=(kc == 0), stop=(kc == NC_D - 1))
            vt = vt_pool.tile([128, VW], F32, tag="vt", name=f"vt{sc}")
            nc.vector.tensor_add(vt[:], ps[:, 0:VW], bvaug_sb[:])
            nc.vector.tensor_scalar_mul(v_sb[:, sc * VW:(sc + 1) * VW],
                                        vt[:], emask_sb[:, sc:sc + 1])

        def proj_pv():
            ps = pool4.tile([128, S], F32, tag="p4", name="ppv")
            for kc in range(NC_D):
                lhsT = pT_sb[:, kc * AT:(kc + 1) * AT]
                nc.tensor.matmul(ps[0:AT, 0:512], lhsT, wv_k[kc][:, 0:512],
                                 start=(kc == 0), stop=(kc == NC_D - 1))
                nc.tensor.matmul(ps[0:AT, 512:VW], lhsT, wv_k[kc][:, 512:VW],
                                 start=(kc == 0), stop=(kc == NC_D - 1))
            pvt = vt_pool.tile([AT, VW], F32, tag="pvt", name="pvt")
            nc.vector.tensor_add(pvt[:], ps[0:AT, 0:VW], bvaug_sb[0:AT, :])
            nc.vector.tensor_mul(pv_sb[0:AT, :], pvt[:], gbc_sb[0:AT, :])
            nc.sync.dma_start(pv_sb[AT:128, :], pv_sb[0:AT, :])

        def scores(h, exp_h):
            c = h // 2
            hp = 64 * (h % 2)
            for tci in range(NC_S):
                st = pool4.tile([128, S], F32, tag="p4",
                                name=f"st_{h}_{tci}")
                lhsT = kT_sb[hp:hp + 64,
                             c * S + tci * 128:c * S + (tci + 1) * 128]
                for sb2 in range(2):
                    nc.tensor.matmul(
                        st[:, sb2 * 512:(sb2 + 1) * 512], lhsT,
                        qT_sb[hp:hp + 64,
                              c * S + sb2 * 512:c * S + (sb2 + 1) * 512],
                        start=True, stop=True, tile_position=(hp, 0))
                nc.scalar.activation(exp_h[:, tci * S:(tci + 1) * S],
                                     st[:], AF.Exp, scale=SCALE)

        def pfx_scores(c):
            ps = pool4.tile([128, S], F32, tag="p4", name=f"pfs_{c}")
            for g in range(2):
                h = 2 * c + g
                hp = 64 * g
                lhsT = pkT_sb[hp:hp + 64, c * AT:(c + 1) * AT]
                for sb2 in range(2):
                    nc.tensor.matmul(
                        ps[hp:hp + 64, sb2 * 512:(sb2 + 1) * 512],
                        lhsT,
                        qT_sb[hp:hp + 64,
                              c * S + sb2 * 512:c * S + (sb2 + 1) * 512],
                        start=True, stop=True, tile_position=(hp, hp))
            pexp = pexp_pool.tile([128, S], BF16, tag="pexp",
                                  name=f"pexp_{c}")
            nc.scalar.activation(pexp[:], ps[:], AF.Exp, scale=SCALE)
            return pexp

        def ctx_tci(h, tci, exp_h, ctxA, ctxB):
            for sc in range(NC_S):
                lhsT = exp_h[:, tci * S + sc * 128:tci * S + (sc + 1) * 128]
                rhs = v_sb[:, tci * VW + h * 65:tci * VW + h * 65 + 65]
                if sc < 7:
                    out = ctxA[:, sc * 65:(sc + 1) * 65]
                    st_fl = (tci == 0 and sc == 0)
                    sp_fl = (tci == NC_S - 1 and sc == 6)
                else:
                    out = ctxB[:, 0:65]
                    st_fl = (tci == 0)
                    sp_fl = (tci == NC_S - 1)
                nc.tensor.matmul(
                    out, lhsT, rhs, start=st_fl, stop=sp_fl,
                    skip_group_check=True)

        def pfx_ctx(h, pexp, pfxP):
            hp = 64 * (h % 2)
            for sc in range(NC_S):
                lhsT = pexp[hp:hp + 64, sc * 128:(sc + 1) * 128]
                rhs = pv_sb[hp:hp + 64, h * 65:h * 65 + 65]
                if sc < 7:
                    out = pfxP[:, sc * 65:(sc + 1) * 65]
                    st_fl, sp_fl = (sc == 0), (sc == 6)
                else:
                    out = pfxP[:, 512:577]
                    st_fl, sp_fl = True, True
                nc.tensor.matmul(out, lhsT, rhs, start=st_fl, stop=sp_fl,
                                 skip_group_check=True,
                                 tile_position=(hp, 0))

        def bcast7(r16, col):
            a = r16[:, col:col + 7]
            return bass.AP(a.tensor, a.offset, [a.ap[0], [1, 7], [0, 64]])

        def strided7(t, off):
            a = t[:]
            return bass.AP(a.tensor, a.offset + off, [a.ap[0], [65, 7]])

        def finish(h, ctxA, ctxB, pfxP):
            r16 = r_pool.tile([128, 16], F32, tag="r16", name=f"r16_{h}")
            cA = ctxA[:, 0:455].rearrange("p (a b) -> p a b", b=65)
            pA = pfxP[:, 0:455].rearrange("p (a b) -> p a b", b=65)
            nc.vector.reciprocal(r16[:, 0:7], strided7(ctxA, 64))
            nc.vector.reciprocal(r16[:, 7:8], ctxB[:, 64:65])
            nc.vector.reciprocal(r16[:, 8:15], strided7(pfxP, 64))
            nc.vector.reciprocal(r16[:, 15:16], pfxP[:, 576:577])
            outb = out_pool.tile([128, 512], F32, tag="ob", name=f"ob_{h}")
            o3 = outb[:].rearrange("p (a b) -> p a b", b=64)
            tmp = out_pool.tile([128, 448], F32, tag="tmp", name=f"tm_{h}")
            t3 = tmp[:].rearrange("p (a b) -> p a b", b=64)
            # prefix reads first: frees the pool4 slot pfxP occupies
            nc.vector.tensor_mul(t3[:, :, :], pA[:, :, 0:64], bcast7(r16, 8))
            nc.vector.tensor_scalar_mul(outb[:, 448:512], ctxB[:, 0:64],
                                        r16[:, 7:8])
            nc.vector.scalar_tensor_tensor(
                outb[:, 448:512], pfxP[:, 512:576], r16[:, 15:16],
                outb[:, 448:512], op0=ALU.mult, op1=ALU.add)
            nc.vector.tensor_mul(o3[:, 0:7, :], cA[:, :, 0:64],
                                 bcast7(r16, 0))
            nc.gpsimd.tensor_add(outb[:, 0:448], outb[:, 0:448], tmp[:])
            base = out_nat[:, :]
            dst = bass.AP(base.tensor, base.offset + h * 64,
                          [[D, 128], [128 * D, 8], [1, 64]])
            nc.sync.dma_start(dst, o3[:, :, :])

        # ---- master emission sequence ----
        proj_qk(0, "q")
        proj_qk(0, "k")
        proj_pk(0)
        proj_v(0)
        proj_v(1)

        pexp_cur = None
        for h in range(H):
            c = h // 2
            exp_h = exp_pool.tile([128, NC_S * S], BF16, tag="exp",
                                  name=f"exp_{h}")
            scores(h, exp_h)
            if h % 2 == 0:
                pexp_cur = pfx_scores(c)
                if c + 1 < NC_D:
                    proj_qk(c + 1, "q")
            else:
                if c + 1 < NC_D:
                    proj_qk(c + 1, "k")
                    proj_pk(c + 1)
            if h == 0:
                proj_pv()
            ctxA = ctx_pool.tile([128, 512], F32, tag="ctx",
                                 name=f"cA_{h}")
            ctxB = ctx_pool.tile([128, 512], F32, tag="ctx",
                                 name=f"cB_{h}")
            for tci in range(NC_S):
                if h == 0 and tci < 6:
                    proj_v(tci + 2)
                ctx_tci(h, tci, exp_h, ctxA, ctxB)
            pfxP = pool4.tile([128, S], F32, tag="p4", name=f"pfxp_{h}")
            pfx_ctx(h, pexp_cur, pfxP)
            finish(h, ctxA, ctxB, pfxP)


def _prep_inputs(hidden_states, prompt_tokens, gating_factor, attention_mask,
                 Wq, bq, Wk, bk, Wv, bv):
    bf = ml_dtypes.bfloat16
    hs = np.asarray(hidden_states, np.float32)
    mask = np.asarray(attention_mask, np.float32).reshape(B, S)
    wqT = np.ascontiguousarray(np.asarray(Wq, np.float32).T).astype(bf)
    wkT = np.ascontiguousarray(np.asarray(Wk, np.float32).T).astype(bf)
    # augmented WvT: [din, 780], col 65h+j = Wv.T[:, 64h+j], col 65h+64 = 0
    wvT_f = np.asarray(Wv, np.float32).T
    wvT_aug = np.zeros((D, VW), np.float32)
    idx = np.arange(D)
    aug_cols = (idx // DH) * (DH + 1) + (idx % DH)
    wvT_aug[:, aug_cols] = wvT_f
    wvT_aug = wvT_aug.astype(bf)
    bq_c = np.asarray(bq, np.float32).reshape(D, 1)
    bk_c = np.asarray(bk, np.float32).reshape(D, 1)
    bv_aug = np.zeros(VW, np.float32)
    bv_aug[aug_cols] = np.asarray(bv, np.float32)
    bv_aug[DH::DH + 1] = 1.0
    bvaug_bc = np.ascontiguousarray(
        np.broadcast_to(bv_aug, (128, VW)), np.float32)
    pT = np.ascontiguousarray(
        np.asarray(prompt_tokens, np.float32)[0].T).astype(bf)
    gat_row = np.repeat(
        np.asarray(gating_factor, np.float32).reshape(H), DH + 1)
    gat = np.ascontiguousarray(
        np.broadcast_to(gat_row, (128, VW)), np.float32)

    shared = dict(wqT=wqT, wkT=wkT, wvT=wvT_aug, bq=bq_c, bk=bk_c,
                  bvaug=bvaug_bc, promptT=pT, gating=gat)
    in_maps = []
    for b in range(B):
        m = dict(shared)
        m["hsT"] = np.ascontiguousarray(hs[b].T).astype(bf)
        m["mask"] = np.ascontiguousarray(mask[b].reshape(S, 1))
        in_maps.append(m)
    return in_maps


def kernel(**inputs):
    global LAST_RESULTS
    if "nc" not in _CACHE:
        _CACHE["nc"] = _build_nc()
    nc = _CACHE["nc"]
    in_maps = _prep_inputs(**inputs)
    res = None
    for attempt in range(3):
        try:
            res = run_bass_kernel_spmd(nc, in_maps, list(range(B)))
            break
        except ModuleNotFoundError:
            import os

            os.environ["BASS_NEVER_TRACE"] = "1"
            if attempt == 2:
                raise
        except Exception:
            if attempt == 2:
                raise
    LAST_RESULTS = res
    out = np.empty((B, S, D), np.float32)
    for b in range(B):
        out[b] = res.results[b]["out_nat"]
    return out


# revision 16
# speedup vs baseline: 1.5204x; 1.0845x over previous
"""BertSelfAttention with gated prompt-prefix branch on 8 Trainium2 cores.

Sharding: data-parallel over batch (B=8 -> 1 batch element per core), no
collectives. Head-granular pipeline per core:

  Q/K proj (bf16, K=128)  -> fp8 q8/k8 in DoubleRow slab layout
  scores_h = k8.T @ q8    fp8 DoubleRow (K=32x2 slots), [t,s], 2x col rate
  exp = exp(SCALE*score)  ACT, bf16 out [t, s]
  ctx accumulates NATURAL [s, d]: stationary = exp[t-block, s-block],
  rhs = v_aug[t-block, 65] (col 64 = ones*e^mask -> denominator lands in
  psum col 64, PER-PARTITION in s)
  prefix branch identical from prompt-derived pk8/pv (tanh(gate) folded
  into pv on-device)
  finish: DVE reciprocal + broadcast-mul + scalar_tensor_tensor, all
  per-partition; output written natural [1024, 768] f32 (no transpose)
"""

import numpy as np
import ml_dtypes

import concourse.bass as bass
import concourse.mybir as mybir
import concourse.tile as tile
from concourse.bass_utils import run_bass_kernel_spmd
from concourse.vector_clock import ScopedClock


class SplitDrainTileContext(tile.TileContext):
    """This walrus build rejects >2 sync waits on the kernel-tail Drain
    ("Too many sync wait commands"); split them across SP nops instead."""

    def _drain_and_barrier(self, tick_clock, wait_clock):
        probe = self.nc.sync.nop(nofuse=True, hint="drain_wait_split")
        wait_clock.add_sem_waits(
            probe.ins, ScopedClock({None: tick_clock.global_clock})
        )
        waits = list(probe.ins.sync_info.on_wait or [])
        if len(waits) > 1:
            probe.ins.sync_info.on_wait = waits[:1]
            for i in range(1, len(waits)):
                extra = self.nc.sync.nop(nofuse=True, hint="drain_wait_split")
                extra.ins.sync_info = mybir.SyncInfo(
                    on_wait=waits[i : i + 1], on_update=[]
                )
        drain_inst = self.nc.sync.drain()
        if drain_inst.ins.sync_info is not None:
            drain_inst.ins.sync_info.on_wait = []
        self.nc.all_engine_barrier()
        assert self.sems is not None
        popped = self.nc._tile_sem_poison_stack.pop()
        assert popped is self._sem_poison
        self.nc.clear_and_free_semaphores(list(self.sems.allocated().values()))
        self.nc.all_engine_barrier()

F32 = mybir.dt.float32
BF16 = mybir.dt.bfloat16
FP8 = mybir.dt.float8e4
AF = mybir.ActivationFunctionType
ALU = mybir.AluOpType
DR = mybir.MatmulPerfMode.DoubleRow

H, DH, D = 12, 64, 768
S, AT, B = 1024, 64, 8
SCALE = 1.0 / np.sqrt(DH)
NC_D = D // 128  # 6 contraction chunks
NC_S = S // 128  # 8 sequence chunks
VW = H * (DH + 1)  # 780: v with per-head ones column

_CACHE = {}
LAST_RESULTS = None


def _split_sync_waits(nc, cap=1):
    """Walrus on this image allows very few sync-wait commands per
    instruction (tensor_scalar rejects 2). Hoist excess waits onto
    same-engine nops placed immediately before the instruction."""
    for bb in nc.main_func.blocks:
        cur = list(bb.instructions)
        out = []
        for inst in cur:
            si = inst.sync_info
            waits = list(si.on_wait) if si and si.on_wait else []
            if len(waits) > cap:
                for i in range(0, len(waits) - cap):
                    bi = nc.engines[inst.engine].nop(
                        nofuse=True, hint="wait_split")
                    popped = nc.cur_bb.bb.instructions.pop()
                    assert popped is bi.ins
                    bi.ins.sync_info = mybir.SyncInfo(
                        on_wait=waits[i : i + 1], on_update=[])
                    out.append(bi.ins)
                si.on_wait = waits[len(waits) - cap:]
            out.append(inst)
        bb.instructions[:] = out


def _build_nc():
    nc = bass.Bass()
    hsT = nc.dram_tensor("hsT", [D, S], BF16, kind="ExternalInput")
    wqT = nc.dram_tensor("wqT", [D, D], BF16, kind="ExternalInput")
    wkT = nc.dram_tensor("wkT", [D, D], BF16, kind="ExternalInput")
    wvT = nc.dram_tensor("wvT", [D, VW], BF16, kind="ExternalInput")
    bq = nc.dram_tensor("bq", [D, 1], F32, kind="ExternalInput")
    bk = nc.dram_tensor("bk", [D, 1], F32, kind="ExternalInput")
    bvaug = nc.dram_tensor("bvaug", [128, VW], F32, kind="ExternalInput")
    promptT = nc.dram_tensor("promptT", [D, AT], BF16, kind="ExternalInput")
    mask = nc.dram_tensor("mask", [S, 1], F32, kind="ExternalInput")
    gating = nc.dram_tensor("gating", [128, VW], F32, kind="ExternalInput")
    out_nat = nc.dram_tensor("out_nat", [S, D], F32, kind="ExternalOutput")

    with SplitDrainTileContext(nc) as tc:
        _emit(nc, tc, hsT, wqT, wkT, wvT, bq, bk, bvaug, promptT, mask,
              gating, out_nat)
    _split_sync_waits(nc)
    return nc


def _emit(nc, tc, hsT, wqT, wkT, wvT, bq, bk, bvaug, promptT, mask, gating,
          out_nat):
    from contextlib import ExitStack

    with ExitStack() as ctx:
        pers = ctx.enter_context(tc.tile_pool(name="pers", bufs=1))

        # ---- persistent SBUF ----
        hs_k = [pers.tile([128, S], BF16, tag=f"hs{k}", name=f"hs{k}")
                for k in range(NC_D)]
        wq_c = [pers.tile([128, D], BF16, tag=f"wq{c}", name=f"wq{c}")
                for c in range(NC_D)]
        wk_c = [pers.tile([128, D], BF16, tag=f"wk{c}", name=f"wk{c}")
                for c in range(NC_D)]
        wv_k = [pers.tile([128, VW], BF16, tag=f"wv{k}", name=f"wv{k}")
                for k in range(NC_D)]
        pT_sb = pers.tile([128, NC_D * AT], BF16, tag="pT")
        bq_sb = pers.tile([128, NC_D], F32, tag="bq")
        bk_sb = pers.tile([128, NC_D], F32, tag="bk")
        bvaug_sb = pers.tile([128, VW], F32, tag="bvaug")
        graw_sb = pers.tile([128, VW], F32, tag="graw")
        gbc_sb = pers.tile([128, VW], F32, tag="gbc")
        mask_sb = pers.tile([128, NC_S], F32, tag="mask")
        emask_sb = pers.tile([128, NC_S], F32, tag="emask")
        # bf16 q/k in d-major layout straight from the projection psum:
        # chunk c holds heads (2c, 2c+1) on partition halves
        qT_sb = pers.tile([128, NC_D * S], BF16, tag="qT")
        kT_sb = pers.tile([128, NC_D * S], BF16, tag="kT")
        pkT_sb = pers.tile([128, NC_D * AT], BF16, tag="pkT")
        v_sb = pers.tile([128, NC_S * VW], BF16, tag="v")
        pv_sb = pers.tile([128, VW], BF16, tag="pv")

        # ---- rotating SBUF pools ----
        exp_pool = ctx.enter_context(tc.tile_pool(name="expp", bufs=3))
        pexp_pool = ctx.enter_context(tc.tile_pool(name="pexpp", bufs=3))
        vt_pool = ctx.enter_context(tc.tile_pool(name="vtp", bufs=2))
        out_pool = ctx.enter_context(tc.tile_pool(name="outp", bufs=2))
        r_pool = ctx.enter_context(tc.tile_pool(name="rp", bufs=2))

        # ---- PSUM: pool4 = 3 tiles x [128,1024] (6 banks);
        #      ctx_pool = 2 tiles x [128,512] (2 banks) ----
        pool4 = ctx.enter_context(
            tc.tile_pool(name="p4", bufs=3, space="PSUM"))
        ctx_pool = ctx.enter_context(
            tc.tile_pool(name="ctxp", bufs=2, space="PSUM"))

        # ---- input DMAs, priority order ----
        for k in range(NC_D):
            nc.sync.dma_start(
                hs_k[k][:], hsT[k * 128:(k + 1) * 128, :])
        for c in (0,):
            nc.sync.dma_start(
                wq_c[c][:].rearrange("p (k n) -> p k n", n=128),
                wqT[:, c * 128:(c + 1) * 128].rearrange(
                    "(k p) n -> p k n", p=128))
            nc.sync.dma_start(
                wk_c[c][:].rearrange("p (k n) -> p k n", n=128),
                wkT[:, c * 128:(c + 1) * 128].rearrange(
                    "(k p) n -> p k n", p=128))
        nc.sync.dma_start(bq_sb[:], bq.rearrange("(c p) 1 -> p c", p=128))
        nc.sync.dma_start(bk_sb[:], bk.rearrange("(c p) 1 -> p c", p=128))
        nc.sync.dma_start(mask_sb[:], mask.rearrange("(c p) 1 -> p c", p=128))
        nc.sync.dma_start(bvaug_sb[:], bvaug[:])
        nc.sync.dma_start(graw_sb[:], gating[:])
        nc.sync.dma_start(
            pT_sb[:].rearrange("p (k n) -> p k n", n=AT),
            promptT[:, :].rearrange("(k p) n -> p k n", p=128))
        for k in range(NC_D):
            nc.sync.dma_start(
                wv_k[k][:], wvT[k * 128:(k + 1) * 128, :])
        for c in range(1, NC_D):
            nc.sync.dma_start(
                wq_c[c][:].rearrange("p (k n) -> p k n", n=128),
                wqT[:, c * 128:(c + 1) * 128].rearrange(
                    "(k p) n -> p k n", p=128))
            nc.sync.dma_start(
                wk_c[c][:].rearrange("p (k n) -> p k n", n=128),
                wkT[:, c * 128:(c + 1) * 128].rearrange(
                    "(k p) n -> p k n", p=128))

        # ---- small precompute ----
        nc.scalar.activation(gbc_sb[:], graw_sb[:], AF.Tanh)
        ones_slots = gbc_sb[:, :].rearrange(
            "p (h e) -> p h e", h=H)[:, :, DH:DH + 1]
        nc.vector.memset(ones_slots, 1.0)
        nc.scalar.activation(emask_sb[:], mask_sb[:], AF.Exp)

        # ---- emission helpers ----
        def proj_qk(c, which):
            w_c, b_sb, dst = ((wq_c, bq_sb, qT_sb) if which == "q"
                              else (wk_c, bk_sb, kT_sb))
            ps = pool4.tile([128, S], F32, tag="p4", name=f"pqk_{c}_{which}")
            for kc in range(NC_D):
                lhsT = w_c[c][:, kc * 128:(kc + 1) * 128]
                for sb2 in range(2):
                    nc.tensor.matmul(
                        ps[:, sb2 * 512:(sb2 + 1) * 512], lhsT,
                        hs_k[kc][:, sb2 * 512:(sb2 + 1) * 512],
                        start=(kc == 0), stop=(kc == NC_D - 1))
            nc.vector.tensor_scalar_add(dst[:, c * S:(c + 1) * S], ps[:],
                                        b_sb[:, c:c + 1])

        def proj_pk(c):
            ps = pool4.tile([128, S], F32, tag="p4", name=f"ppk_{c}")
            for kc in range(NC_D):
                nc.tensor.matmul(
                    ps[:, 0:AT],
                    wk_c[c][:, kc * 128:(kc + 1) * 128],
                    pT_sb[:, kc * AT:(kc + 1) * AT],
                    start=(kc == 0), stop=(kc == NC_D - 1))
            nc.vector.tensor_scalar_add(pkT_sb[:, c * AT:(c + 1) * AT],
                                        ps[:, 0:AT], bk_sb[:, c:c + 1])

        def proj_v(sc):
            ps = pool4.tile([128, S], F32, tag="p4", name=f"pv_{sc}")
            for kc in range(NC_D):
                lhsT = hs_k[kc][:, sc * 128:(sc + 1) * 128]
                nc.tensor.matmul(ps[:, 0:512], lhsT, wv_k[kc][:, 0:512],
                                 start=(kc == 0), stop=(kc == NC_D - 1))
                nc.tensor.matmul(ps[:, 512:VW], lhsT, wv_k[kc][:, 512:VW],
                                 start=(kc == 0), stop=(kc == NC_D - 1))
            vt = vt_pool.tile([128, VW], F32, tag="vt", name=f"vt{sc}")
            nc.vector.tensor_add(vt[:], ps[:, 0:VW], bvaug_sb[:])
            nc.vector.tensor_scalar_mul(v_sb[:, sc * VW:(sc + 1) * VW],
                                        vt[:], emask_sb[:, sc:sc + 1])

        def proj_pv():
            ps = pool4.tile([128, S], F32, tag="p4", name="ppv")
            for kc in range(NC_D):
                lhsT = pT_sb[:, kc * AT:(kc + 1) * AT]
                nc.tensor.matmul(ps[0:AT, 0:512], lhsT, wv_k[kc][:, 0:512],
                                 start=(kc == 0), stop=(kc == NC_D - 1))
                nc.tensor.matmul(ps[0:AT, 512:VW], lhsT, wv_k[kc][:, 512:VW],
                                 start=(kc == 0), stop=(kc == NC_D - 1))
            pvt = vt_pool.tile([AT, VW], F32, tag="pvt", name="pvt")
            nc.vector.tensor_add(pvt[:], ps[0:AT, 0:VW], bvaug_sb[0:AT, :])
            nc.vector.tensor_mul(pv_sb[0:AT, :], pvt[:], gbc_sb[0:AT, :])
            nc.sync.dma_start(pv_sb[AT:128, :], pv_sb[0:AT, :])

        def scores_tci(h, tci, exp_h):
            c = h // 2
            hp = 64 * (h % 2)
            st = pool4.tile([128, S], F32, tag="p4",
                            name=f"st_{h}_{tci}")
            lhsT = kT_sb[hp:hp + 64,
                         c * S + tci * 128:c * S + (tci + 1) * 128]
            for sb2 in range(2):
                nc.tensor.matmul(
                    st[:, sb2 * 512:(sb2 + 1) * 512], lhsT,
                    qT_sb[hp:hp + 64,
                          c * S + sb2 * 512:c * S + (sb2 + 1) * 512],
                    start=True, stop=True, tile_position=(hp, 0))
            nc.scalar.activation(exp_h[:, tci * S:(tci + 1) * S],
                                 st[:], AF.Exp, scale=SCALE)

        def pfx_scores(c):
            ps = pool4.tile([128, S], F32, tag="p4", name=f"pfs_{c}")
            for g in range(2):
                h = 2 * c + g
                hp = 64 * g
                lhsT = pkT_sb[hp:hp + 64, c * AT:(c + 1) * AT]
                for sb2 in range(2):
                    nc.tensor.matmul(
                        ps[hp:hp + 64, sb2 * 512:(sb2 + 1) * 512],
                        lhsT,
                        qT_sb[hp:hp + 64,
                              c * S + sb2 * 512:c * S + (sb2 + 1) * 512],
                        start=True, stop=True, tile_position=(hp, hp))
            pexp = pexp_pool.tile([128, S], BF16, tag="pexp",
                                  name=f"pexp_{c}")
            nc.scalar.activation(pexp[:], ps[:], AF.Exp, scale=SCALE)
            return pexp

        def ctx_tci(h, tci, exp_h, ctxA, ctxB):
            for sc in range(NC_S):
                lhsT = exp_h[:, tci * S + sc * 128:tci * S + (sc + 1) * 128]
                rhs = v_sb[:, tci * VW + h * 65:tci * VW + h * 65 + 65]
                if sc < 7:
                    out = ctxA[:, sc * 65:(sc + 1) * 65]
                    st_fl = (tci == 0 and sc == 0)
                    sp_fl = (tci == NC_S - 1 and sc == 6)
                else:
                    out = ctxB[:, 0:65]
                    st_fl = (tci == 0)
                    sp_fl = (tci == NC_S - 1)
                nc.tensor.matmul(
                    out, lhsT, rhs, start=st_fl, stop=sp_fl,
                    skip_group_check=True)

        def pfx_ctx(h, pexp, pfxP):
            hp = 64 * (h % 2)
            for sc in range(NC_S):
                lhsT = pexp[hp:hp + 64, sc * 128:(sc + 1) * 128]
                rhs = pv_sb[hp:hp + 64, h * 65:h * 65 + 65]
                if sc < 7:
                    out = pfxP[:, sc * 65:(sc + 1) * 65]
                    st_fl, sp_fl = (sc == 0), (sc == 6)
                else:
                    out = pfxP[:, 512:577]
                    st_fl, sp_fl = True, True
                nc.tensor.matmul(out, lhsT, rhs, start=st_fl, stop=sp_fl,
                                 skip_group_check=True,
                                 tile_position=(hp, 0))

        def bcast7(r16, col):
            a = r16[:, col:col + 7]
            return bass.AP(a.tensor, a.offset, [a.ap[0], [1, 7], [0, 64]])

        def strided7(t, off):
            a = t[:]
            return bass.AP(a.tensor, a.offset + off, [a.ap[0], [65, 7]])

        def finish(h, ctxA, ctxB, pfxP):
            r16 = r_pool.tile([128, 16], F32, tag="r16", name=f"r16_{h}")
            cA = ctxA[:, 0:455].rearrange("p (a b) -> p a b", b=65)
            pA = pfxP[:, 0:455].rearrange("p (a b) -> p a b", b=65)
            nc.vector.reciprocal(r16[:, 0:7], strided7(ctxA, 64))
            nc.vector.reciprocal(r16[:, 7:8], ctxB[:, 64:65])
            nc.vector.reciprocal(r16[:, 8:15], strided7(pfxP, 64))
            nc.vector.reciprocal(r16[:, 15:16], pfxP[:, 576:577])
            outb = out_pool.tile([128, 512], F32, tag="ob", name=f"ob_{h}")
            o3 = outb[:].rearrange("p (a b) -> p a b", b=64)
            tmp = out_pool.tile([128, 448], F32, tag="tmp", name=f"tm_{h}")
            t3 = tmp[:].rearrange("p (a b) -> p a b", b=64)
            # prefix reads first: frees the pool4 slot pfxP occupies
            nc.vector.tensor_mul(t3[:, :, :], pA[:, :, 0:64], bcast7(r16, 8))
            nc.vector.tensor_scalar_mul(outb[:, 448:512], ctxB[:, 0:64],
                                        r16[:, 7:8])
            nc.vector.scalar_tensor_tensor(
                outb[:, 448:512], pfxP[:, 512:576], r16[:, 15:16],
                outb[:, 448:512], op0=ALU.mult, op1=ALU.add)
            nc.vector.tensor_mul(o3[:, 0:7, :], cA[:, :, 0:64],
                                 bcast7(r16, 0))
            nc.gpsimd.tensor_add(outb[:, 0:448], outb[:, 0:448], tmp[:])
            base = out_nat[:, :]
            dst = bass.AP(base.tensor, base.offset + h * 64,
                          [[D, 128], [128 * D, 8], [1, 64]])
            nc.sync.dma_start(dst, o3[:, :, :])

        # ---- master emission sequence ----
        proj_qk(0, "q")
        proj_qk(0, "k")
        proj_pk(0)
        proj_v(0)
        proj_v(1)

        pexp_cur = None
        for h in range(H):
            c = h // 2
            exp_h = exp_pool.tile([128, NC_S * S], BF16, tag="exp",
                                  name=f"exp_{h}")
            scores_tci(h, 0, exp_h)
            # proj unit early in the head: its psum slot is free and the
            # scheduler can overlap proj matmuls with the EXP-paced scores
            if h % 2 == 0 and c + 1 < NC_D:
                proj_qk(c + 1, "q")
            if h % 2 == 1 and c + 1 < NC_D:
                proj_qk(c + 1, "k")
            for tci in range(1, NC_S):
                scores_tci(h, tci, exp_h)
            if h % 2 == 0:
                pexp_cur = pfx_scores(c)
            else:
                if c + 1 < NC_D:
                    proj_pk(c + 1)
            if h == 0:
                proj_pv()
            ctxA = ctx_pool.tile([128, 512], F32, tag="ctx",
                                 name=f"cA_{h}")
            ctxB = ctx_pool.tile([128, 512], F32, tag="ctx",
                                 name=f"cB_{h}")
            for tci in range(NC_S):
                if h == 0 and tci < 6:
                    proj_v(tci + 2)
                ctx_tci(h, tci, exp_h, ctxA, ctxB)
            pfxP = pool4.tile([128, S], F32, tag="p4", name=f"pfxp_{h}")
            pfx_ctx(h, pexp_cur, pfxP)
            finish(h, ctxA, ctxB, pfxP)


def _prep_inputs(hidden_states, prompt_tokens, gating_factor, attention_mask,
                 Wq, bq, Wk, bk, Wv, bv):
    bf = ml_dtypes.bfloat16
    hs = np.asarray(hidden_states, np.float32)
    mask = np.asarray(attention_mask, np.float32).reshape(B, S)
    wqT = np.ascontiguousarray(np.asarray(Wq, np.float32).T).astype(bf)
    wkT = np.ascontiguousarray(np.asarray(Wk, np.float32).T).astype(bf)
    # augmented WvT: [din, 780], col 65h+j = Wv.T[:, 64h+j], col 65h+64 = 0
    wvT_f = np.asarray(Wv, np.float32).T
    wvT_aug = np.zeros((D, VW), np.float32)
    idx = np.arange(D)
    aug_cols = (idx // DH) * (DH + 1) + (idx % DH)
    wvT_aug[:, aug_cols] = wvT_f
    wvT_aug = wvT_aug.astype(bf)
    bq_c = np.asarray(bq, np.float32).reshape(D, 1)
    bk_c = np.asarray(bk, np.float32).reshape(D, 1)
    bv_aug = np.zeros(VW, np.float32)
    bv_aug[aug_cols] = np.asarray(bv, np.float32)
    bv_aug[DH::DH + 1] = 1.0
    bvaug_bc = np.ascontiguousarray(
        np.broadcast_to(bv_aug, (128, VW)), np.float32)
    pT = np.ascontiguousarray(
        np.asarray(prompt_tokens, np.float32)[0].T).astype(bf)
    gat_row = np.repeat(
        np.asarray(gating_factor, np.float32).reshape(H), DH + 1)
    gat = np.ascontiguousarray(
        np.broadcast_to(gat_row, (128, VW)), np.float32)

    shared = dict(wqT=wqT, wkT=wkT, wvT=wvT_aug, bq=bq_c, bk=bk_c,
                  bvaug=bvaug_bc, promptT=pT, gating=gat)
    in_maps = []
    for b in range(B):
        m = dict(shared)
        m["hsT"] = np.ascontiguousarray(hs[b].T).astype(bf)
        m["mask"] = np.ascontiguousarray(mask[b].reshape(S, 1))
        in_maps.append(m)
    return in_maps


def kernel(**inputs):
    global LAST_RESULTS
    if "nc" not in _CACHE:
        _CACHE["nc"] = _build_nc()
    nc = _CACHE["nc"]
    in_maps = _prep_inputs(**inputs)
    res = None
    for attempt in range(3):
        try:
            res = run_bass_kernel_spmd(nc, in_maps, list(range(B)))
            break
        except ModuleNotFoundError:
            import os

            os.environ["BASS_NEVER_TRACE"] = "1"
            if attempt == 2:
                raise
        except Exception:
            if attempt == 2:
                raise
    LAST_RESULTS = res
    out = np.empty((B, S, D), np.float32)
    for b in range(B):
        out[b] = res.results[b]["out_nat"]
    return out
